# revision 17
# baseline (speedup 1.0000x reference)
"""JointAtt (dense_cnn) Trainium2 Bass kernel — v5 (GpSimd-free, 2-slice batch).

Per core: 8 slices (n,g) of x [128, 4096] fp16, processed as 4 groups of 2.
Group layout: slice i of a group owns partition band b=64i (PE matmul
tile_position cols {0,64}), so the pooling octaves of both slices live in
ONE PSUM tile P2 [128, 2, 512] and are folded by two DVE tensor_reduce ops
— no GpSimd trees (v3's trees contended with the DVE's SBUF ports and
serialized the whole kernel).

  PE:     ~3.4us of warmup matmuls while the first x load is in flight
          (HAM un-throttle: cold PE runs at 1.2 GHz, warm at 2.4);
          per slice 16 accumulating conv matmuls, all with contiguous or
          j-inner moving APs (216 ns each warm; a w-major moving AP would
          make consecutive columns 128B apart and halve the stream rate);
          2 attention matmuls per slice on row-tile b (whw replicated per
          band; each slice's logits in their OWN PSUM bank — concurrent
          row tiles sharing a bank is a HW hazard).
  DVE:    2 tensor_reduce folds per group (PSUM->SBUF, FD=512, the w-fold
          via a strided view — 1x mode doesn't care); 1 hswish STT per
          group; per slice 2 big fp16 2x-rate TTs OUT = X * ahe * aw.
  Scalar: hswish Relu and T-3 Copy, sigmoids (AHE with broadcast width-2
          trick keeps the DVE multiply at 2x), store DMA triggers.
  DMA:    1 contiguous 1 MB load (sync ring) + 1 MB store (scalar ring)
          per slice; channel shuffle + fp32 conversion on the host.
"""

import numpy as np

import concourse.bass as bass
import concourse.bacc as bacc
import concourse.mybir as mybir
import concourse.tile as tile
from concourse.bass_utils import run_bass_kernel_spmd

F32 = mybir.dt.float32
F16 = mybir.dt.float16

N_CORES = 8
NB = 2          # batches per core
C = 512
G = 4           # groups (of channels, in the model)
CG = 128        # channels per group
H = 64
W = 64
HW = H * W
S = NB * G      # slices per core
GRP = 2         # slices per partition-batched group
MIP = 16        # conv1 output channels
J = 8           # pooling octave width
EPS = 1e-5

_NC_CACHE = None


def _build_bass():
    nc = bacc.Bacc(None, target_bir_lowering=False)

    x_d = nc.dram_tensor("x", [S, CG, HW], F16, kind="ExternalInput")
    whwc_d = nc.dram_tensor("whwc", [CG, 2 * CG + MIP], F16, kind="ExternalInput")
    bact4_d = nc.dram_tensor("bact4", [CG, 1], F32, kind="ExternalInput")
    bhw_d = nc.dram_tensor("bhw", [CG, 2], F32, kind="ExternalInput")
    out_d = nc.dram_tensor("out", [S, CG, HW], F16, kind="ExternalOutput")

    Relu = mybir.ActivationFunctionType.Relu
    Copy = mybir.ActivationFunctionType.Copy
    Sigmoid = mybir.ActivationFunctionType.Sigmoid
    ADD = mybir.AluOpType.add
    MIN = mybir.AluOpType.min
    MULT = mybir.AluOpType.mult

    with tile.TileContext(nc) as tc:
        with (
            tc.tile_pool(name="consts", bufs=1) as consts,
            tc.tile_pool(name="xp", bufs=8) as xp,
            tc.tile_pool(name="op", bufs=4) as op,
            tc.tile_pool(name="ps", bufs=1, space="PSUM") as ps,
            tc.tile_pool(name="sm", bufs=2) as sm,
        ):
            # one packed fp16 const DMA rides the sync ring FIRST
            # (sync-ring DMAs complete ~4us earlier than scalar-ring ones,
            # whose stream sits behind the ACT table load); the x loads
            # follow on sync with only one trigger ahead of them.
            whwc = consts.tile([CG, 2 * CG + MIP], F16)
            nc.sync.dma_start(out=whwc, in_=whwc_d[:])
            whw4 = whwc[:, 0 : 2 * CG]
            w1t = whwc[:, 2 * CG :]
            bact4 = consts.tile([CG, 1], F32)
            nc.scalar.dma_start(out=bact4, in_=bact4_d[:])
            bhw = consts.tile([CG, 2], F32)
            nc.scalar.dma_start(out=bhw, in_=bhw_d[:])
            bh = bhw[:, 0:1]
            bw = bhw[:, 1:2]

            # preload the Sigmoid ACT table off the critical path (first
            # sigmoid use otherwise pays a ~1.3us mid-kernel table load)
            sigp = sm.tile([CG, 2], F16, name="sigp", tag="sigp")
            nc.scalar.activation(out=sigp, in_=whwc[:, 0:2], func=Sigmoid)

            # ---- HAM warmup: ~3.5us of junk N=256 matmuls spanning until
            # the first x load lands, so the PE clock gate (cold 1.2 GHz)
            # opens before the first real conv matmul.  Slice 0's octaves
            # later overwrite the region (start=True resets accumulation).
            PWw = ps.tile([CG, HW // J], F32, name="PW", tag="PW", bufs=2)
            for k in range(20):
                nc.tensor.matmul(
                    PWw[:, 0 : 2 * CG],
                    whw4[:, 0:CG],
                    whw4,
                    start=True,
                    stop=True,
                    tile_position=(0, 0),
                )

            # group sizes: singles first so the DVE's multiply stream (the
            # critical resource) starts as early as possible.
            sizes = [1, 1, 2, 2, 2]
            s = 0
            for grp in sizes:
                # ---- conv1+pooling octaves for `grp` slices.  Slice i owns
                # band b=64i (PE col tiles).  h-part octaves in PH (layout
                # (h j), j = w octave), w-part in PW (layout (j w), j = h
                # octave).  Separate tiles so the h-fold only depends on the
                # h-pass matmuls (tile-granular dependency tracking).
                PH = ps.tile([CG, HW // J], F32, name="PH", tag="PH", bufs=2)
                PW = ps.tile([CG, HW // J], F32, name="PW", tag="PW", bufs=2)
                Xs = []
                for i in range(grp):
                    X = xp.tile([CG, HW], F16, name="X")
                    nc.sync.dma_start(out=X, in_=x_d[s + i])
                    Xs.append(X)
                # all h-passes first so the h-fold unblocks earliest
                for i in range(grp):
                    b = 64 * i
                    Xr = Xs[i].rearrange("p (h w) -> p h w", h=H)
                    for k in range(HW // (H * J)):
                        nc.tensor.matmul(
                            PH[b : b + MIP, :],
                            w1t,
                            Xr[:, :, J * k : J * (k + 1)],
                            start=(k == 0),
                            stop=(k == HW // (H * J) - 1),
                            tile_position=(0, b),
                        )
                for i in range(grp):
                    b = 64 * i
                    X = Xs[i]
                    for k in range(HW // (W * J)):
                        nc.tensor.matmul(
                            PW[b : b + MIP, :],
                            w1t,
                            X[:, J * W * k : J * W * (k + 1)],
                            start=(k == 0),
                            stop=(k == HW // (W * J) - 1),
                            tile_position=(0, b),
                        )

                # ---- skinny chains, split by direction: TT1 = X*ahe only
                # needs the h-chain, so it starts while the w-chain is still
                # in flight (hidden under TT1 on scalar/PE).
                APs2 = ps.tile([CG, 2, 512], F32, name="APs2", tag="APs2", bufs=2)
                OUTs = []
                with tc.high_priority(offset=60):
                    # h-chain: fold, hswish, attention, sigmoid (AHE at
                    # width TWO: stride-0 middle dim keeps TT1 at 2x rate)
                    YH = sm.tile([CG, H], F32, name="YH", tag="YH")
                    nc.vector.tensor_reduce(
                        out=YH,
                        in_=PH.rearrange("p (h j) -> p h j", j=J),
                        axis=mybir.AxisListType.X,
                        op=ADD,
                    )
                    T2H = sm.tile([CG, H], F32, name="T2H", tag="T2H")
                    nc.scalar.activation(out=T2H, in_=YH, func=Relu, bias=bact4)
                    T3H = sm.tile([CG, H], F32, name="T3H", tag="T3H")
                    nc.scalar.activation(out=T3H, in_=T2H, func=Copy, bias=-3.0)
                    HSH = sm.tile([CG, H], F16, name="HSH", tag="HSH")
                    nc.vector.scalar_tensor_tensor(
                        out=HSH, in0=T2H, scalar=6.0, in1=T3H, op0=MIN, op1=MULT
                    )
                    for i in range(grp):
                        b = 64 * i
                        nc.tensor.matmul(
                            APs2[:, i : i + 1, 0:H],
                            whw4[b : b + MIP, 0:CG],
                            HSH[b : b + MIP, :].unsqueeze(1),
                            start=True,
                            stop=True,
                            tile_position=(b, 0),
                        )
                    AHE2 = sm.tile([CG, 2, H, 2], F16, name="AHE2", tag="AHE2")
                    nc.scalar.activation(
                        out=AHE2,
                        in_=APs2[:, :, 0:H].unsqueeze(3).broadcast_to([CG, 2, H, 2]),
                        func=Sigmoid,
                        bias=bh,
                    )

                # first TT1 as early as possible
                OUT = op.tile([CG, HW], F16, name="OUT")
                OUTs.append(OUT)
                Xr4 = Xs[0].rearrange("p (h r two) -> p h r two", h=H, two=2)
                OUTr4 = OUT.rearrange("p (h r two) -> p h r two", h=H, two=2)
                ahe_b = AHE2[:, 0].unsqueeze(2).broadcast_to([CG, H, W // 2, 2])
                nc.vector.tensor_tensor(out=OUTr4, in0=Xr4, in1=ahe_b, op=MULT)

                with tc.high_priority(offset=60):
                    # w-chain (runs under TT1)
                    YW = sm.tile([CG, W], F32, name="YW", tag="YW")
                    nc.vector.tensor_reduce(
                        out=YW,
                        in_=PW.rearrange("p (j w) -> p w j", j=J),
                        axis=mybir.AxisListType.X,
                        op=ADD,
                    )
                    T2W = sm.tile([CG, W], F32, name="T2W", tag="T2W")
                    nc.scalar.activation(out=T2W, in_=YW, func=Relu, bias=bact4)
                    T3W = sm.tile([CG, W], F32, name="T3W", tag="T3W")
                    nc.scalar.activation(out=T3W, in_=T2W, func=Copy, bias=-3.0)
                    HSW = sm.tile([CG, W], F16, name="HSW", tag="HSW")
                    nc.vector.scalar_tensor_tensor(
                        out=HSW, in0=T2W, scalar=6.0, in1=T3W, op0=MIN, op1=MULT
                    )
                    for i in range(grp):
                        b = 64 * i
                        nc.tensor.matmul(
                            APs2[:, i : i + 1, H : 2 * H],
                            whw4[b : b + MIP, CG:],
                            HSW[b : b + MIP, :].unsqueeze(1),
                            start=True,
                            stop=True,
                            tile_position=(b, 0),
                        )
                    AW2 = sm.tile([CG, 2, W], F16, name="AW2", tag="AW2")
                    nc.scalar.activation(
                        out=AW2, in_=APs2[:, :, H : 2 * H], func=Sigmoid, bias=bw
                    )

                # remaining TT1s, then TT2 + store per slice
                for i in range(1, grp):
                    OUT = op.tile([CG, HW], F16, name="OUT")
                    OUTs.append(OUT)
                    Xr4 = Xs[i].rearrange("p (h r two) -> p h r two", h=H, two=2)
                    OUTr4 = OUT.rearrange("p (h r two) -> p h r two", h=H, two=2)
                    ahe_b = AHE2[:, i].unsqueeze(2).broadcast_to([CG, H, W // 2, 2])
                    nc.vector.tensor_tensor(out=OUTr4, in0=Xr4, in1=ahe_b, op=MULT)
                for i in range(grp):
                    OUT = OUTs[i]
                    OUTr = OUT.rearrange("p (h w) -> p h w", h=H)
                    aw_b = AW2[:, i].unsqueeze(1).broadcast_to([CG, H, W])
                    nc.vector.tensor_tensor(out=OUTr, in0=OUTr, in1=aw_b, op=MULT)
                    nc.scalar.dma_start(out=out_d[s + i], in_=OUT)
                s += grp

    nc.finalize()
    return nc


def _get_nc():
    global _NC_CACHE
    if _NC_CACHE is None:
        _NC_CACHE = _build_bass()
    return _NC_CACHE


def _prep_weights(W1, b1, gamma, beta, mean, var, Wh, bh, Ww, bw):
    W1 = np.asarray(W1, np.float64)
    b1 = np.asarray(b1, np.float64)
    gamma = np.asarray(gamma, np.float64)
    beta = np.asarray(beta, np.float64)
    mean = np.asarray(mean, np.float64)
    var = np.asarray(var, np.float64)
    Wh = np.asarray(Wh, np.float64)
    Ww = np.asarray(Ww, np.float64)
    bh = np.asarray(bh, np.float64)
    bw = np.asarray(bw, np.float64)

    scale = gamma / np.sqrt(var + EPS)                    # (MIP,)
    w1eff = (W1 * scale[:, None]) / float(W)              # (MIP, CG); mean 1/64
    b1eff = scale * (b1 - mean) + beta                    # (MIP,)

    whw = np.concatenate([(Wh / 6.0).T, (Ww / 6.0).T], axis=1)        # (MIP, 2CG)
    whwc = np.zeros((CG, 2 * CG + MIP), np.float16)
    bact4 = np.zeros((CG, 1), np.float32)
    for b in range(0, CG, 32):
        whwc[b : b + MIP, 0 : 2 * CG] = whw.astype(np.float16)
        bact4[b : b + MIP, 0] = (b1eff + 3.0).astype(np.float32)
    whwc[:, 2 * CG :] = w1eff.T.astype(np.float16)        # (CG, MIP)
    bhw = np.ascontiguousarray(
        np.stack([bh, bw], axis=1).astype(np.float32)
    )                                                     # (CG, 2)
    return whwc, bact4, bhw


def run(inputs: dict, trace: bool = False):
    """Run on 8 NeuronCores. Returns (out [16,512,64,64] fp32, results)."""
    x = np.asarray(inputs["x"], dtype=np.float32)
    n = x.shape[0]
    assert x.shape == (n, C, H, W) and n == N_CORES * NB, x.shape

    whwc, bact4, bhw = _prep_weights(
        inputs["W1"], inputs["b1"], inputs["gamma"], inputs["beta"],
        inputs["mean"], inputs["var"], inputs["Wh"], inputs["bh"],
        inputs["Ww"], inputs["bw"],
    )

    # fp16, pre-sliced per core: [core, slice(b,g), 128, 4096]
    x16 = np.ascontiguousarray(
        x.astype(np.float16).reshape(N_CORES, S, CG, HW)
    )

    nc = _get_nc()
    core_ids = list(range(N_CORES))
    in_maps = []
    for k in core_ids:
        in_maps.append(
            {
                "x": x16[k],
                "whwc": whwc,
                "bact4": bact4,
                "bhw": bhw,
            }
        )

    res = run_bass_kernel_spmd(nc, in_maps, core_ids, trace=trace)
    out16 = np.stack([res.results[k]["out"] for k in core_ids])  # (8,8,128,HW)
    # group-major == natural channel order; then apply the channel shuffle
    # c' = (c % 4) * 128 + c // 4 on the host, with the fp16->fp32 upcast.
    nat = out16.astype(np.float32).reshape(n, C, H, W)
    out = np.ascontiguousarray(
        nat.reshape(n, CG, G, H, W).transpose(0, 2, 1, 3, 4).reshape(n, C, H, W)
    )
    return out, res


def kernel(**inputs) -> np.ndarray:
    out, _ = run(inputs, trace=False)
    return out


def exec_time_ns(res):
    return res.exec_time_ns


# revision 18
# speedup vs baseline: 1.0036x; 1.0036x over previous
"""JointAtt (dense_cnn) Trainium2 Bass kernel — v5 (GpSimd-free, 2-slice batch).

Per core: 8 slices (n,g) of x [128, 4096] fp16, processed as 4 groups of 2.
Group layout: slice i of a group owns partition band b=64i (PE matmul
tile_position cols {0,64}), so the pooling octaves of both slices live in
ONE PSUM tile P2 [128, 2, 512] and are folded by two DVE tensor_reduce ops
— no GpSimd trees (v3's trees contended with the DVE's SBUF ports and
serialized the whole kernel).

  PE:     ~3.4us of warmup matmuls while the first x load is in flight
          (HAM un-throttle: cold PE runs at 1.2 GHz, warm at 2.4);
          per slice 16 accumulating conv matmuls, all with contiguous or
          j-inner moving APs (216 ns each warm; a w-major moving AP would
          make consecutive columns 128B apart and halve the stream rate);
          2 attention matmuls per slice on row-tile b (whw replicated per
          band; each slice's logits in their OWN PSUM bank — concurrent
          row tiles sharing a bank is a HW hazard).
  DVE:    2 tensor_reduce folds per group (PSUM->SBUF, FD=512, the w-fold
          via a strided view — 1x mode doesn't care); 1 hswish STT per
          group; per slice 2 big fp16 2x-rate TTs OUT = X * ahe * aw.
  Scalar: hswish Relu and T-3 Copy, sigmoids (AHE with broadcast width-2
          trick keeps the DVE multiply at 2x), store DMA triggers.
  DMA:    1 contiguous 1 MB load (sync ring) + 1 MB store (scalar ring)
          per slice; channel shuffle + fp32 conversion on the host.
"""

import numpy as np

import concourse.bass as bass
import concourse.bacc as bacc
import concourse.mybir as mybir
import concourse.tile as tile
from concourse.bass_utils import run_bass_kernel_spmd

F32 = mybir.dt.float32
F16 = mybir.dt.float16

N_CORES = 8
NB = 2          # batches per core
C = 512
G = 4           # groups (of channels, in the model)
CG = 128        # channels per group
H = 64
W = 64
HW = H * W
S = NB * G      # slices per core
GRP = 2         # slices per partition-batched group
MIP = 16        # conv1 output channels
J = 4           # pooling octave width
EPS = 1e-5

_NC_CACHE = None


def _build_bass():
    nc = bacc.Bacc(None, target_bir_lowering=False)

    x_d = nc.dram_tensor("x", [S, CG, HW], F16, kind="ExternalInput")
    whwc_d = nc.dram_tensor("whwc", [CG, 2 * CG + MIP], F16, kind="ExternalInput")
    bact4_d = nc.dram_tensor("bact4", [CG, 1], F32, kind="ExternalInput")
    bhw_d = nc.dram_tensor("bhw", [CG, 2], F32, kind="ExternalInput")
    out_d = nc.dram_tensor("out", [S, CG, HW], F16, kind="ExternalOutput")

    Relu = mybir.ActivationFunctionType.Relu
    Copy = mybir.ActivationFunctionType.Copy
    Sigmoid = mybir.ActivationFunctionType.Sigmoid
    ADD = mybir.AluOpType.add
    MIN = mybir.AluOpType.min
    MULT = mybir.AluOpType.mult

    with tile.TileContext(nc) as tc:
        with (
            tc.tile_pool(name="consts", bufs=1) as consts,
            tc.tile_pool(name="xp", bufs=8) as xp,
            tc.tile_pool(name="op", bufs=4) as op,
            tc.tile_pool(name="ps", bufs=1, space="PSUM") as ps,
            tc.tile_pool(name="sm", bufs=2) as sm,
        ):
            # one packed fp16 const DMA rides the sync ring FIRST
            # (sync-ring DMAs complete ~4us earlier than scalar-ring ones,
            # whose stream sits behind the ACT table load); the x loads
            # follow on sync with only one trigger ahead of them.
            whwc = consts.tile([CG, 2 * CG + MIP], F16)
            nc.sync.dma_start(out=whwc, in_=whwc_d[:])
            whw4 = whwc[:, 0 : 2 * CG]
            w1t = whwc[:, 2 * CG :]
            bact4 = consts.tile([CG, 1], F32)
            nc.scalar.dma_start(out=bact4, in_=bact4_d[:])
            bhw = consts.tile([CG, 2], F32)
            nc.scalar.dma_start(out=bhw, in_=bhw_d[:])
            bh = bhw[:, 0:1]
            bw = bhw[:, 1:2]

            # preload the Sigmoid ACT table off the critical path (first
            # sigmoid use otherwise pays a ~1.3us mid-kernel table load)
            sigp = sm.tile([CG, 2], F16, name="sigp", tag="sigp")
            nc.scalar.activation(out=sigp, in_=whwc[:, 0:2], func=Sigmoid)

            # ---- HAM warmup: ~3.5us of junk N=256 matmuls spanning until
            # the first x load lands, so the PE clock gate (cold 1.2 GHz)
            # opens before the first real conv matmul.  Slice 0's octaves
            # later overwrite the region (start=True resets accumulation).
            PWw = ps.tile([CG, H * J], F32, name="PW", tag="PW", bufs=2)
            for k in range(20):
                nc.tensor.matmul(
                    PWw[:, 0 : 2 * CG],
                    whw4[:, 0:CG],
                    whw4,
                    start=True,
                    stop=True,
                    tile_position=(0, 0),
                )

            # group sizes: singles first so the DVE's multiply stream (the
            # critical resource) starts as early as possible.
            sizes = [1, 1, 2, 2, 2]
            s = 0
            for grp in sizes:
                # ---- conv1+pooling octaves for `grp` slices.  Slice i owns
                # band b=64i (PE col tiles).  h-part octaves in PH (layout
                # (h j), j = w octave), w-part in PW (layout (j w), j = h
                # octave).  Separate tiles so the h-fold only depends on the
                # h-pass matmuls (tile-granular dependency tracking).
                PH = ps.tile([CG, H * J], F32, name="PH", tag="PH", bufs=2)
                PW = ps.tile([CG, H * J], F32, name="PW", tag="PW", bufs=2)
                Xs = []
                for i in range(grp):
                    X = xp.tile([CG, HW], F16, name="X")
                    nc.sync.dma_start(out=X, in_=x_d[s + i])
                    Xs.append(X)
                # all h-passes first so the h-fold unblocks earliest
                for i in range(grp):
                    b = 64 * i
                    Xr = Xs[i].rearrange("p (h w) -> p h w", h=H)
                    for k in range(HW // (H * J)):
                        nc.tensor.matmul(
                            PH[b : b + MIP, :],
                            w1t,
                            Xr[:, :, J * k : J * (k + 1)],
                            start=(k == 0),
                            stop=(k == HW // (H * J) - 1),
                            tile_position=(0, b),
                        )
                for i in range(grp):
                    b = 64 * i
                    X = Xs[i]
                    for k in range(HW // (W * J)):
                        nc.tensor.matmul(
                            PW[b : b + MIP, :],
                            w1t,
                            X[:, J * W * k : J * W * (k + 1)],
                            start=(k == 0),
                            stop=(k == HW // (W * J) - 1),
                            tile_position=(0, b),
                        )

                # ---- skinny chains, split by direction: TT1 = X*ahe only
                # needs the h-chain, so it starts while the w-chain is still
                # in flight (hidden under TT1 on scalar/PE).
                APs2 = ps.tile([CG, 2, 512], F32, name="APs2", tag="APs2", bufs=2)
                OUTs = []
                with tc.high_priority(offset=60):
                    # h-chain: fold, hswish, attention, sigmoid (AHE at
                    # width TWO: stride-0 middle dim keeps TT1 at 2x rate)
                    YH = sm.tile([CG, H], F32, name="YH", tag="YH")
                    nc.vector.tensor_reduce(
                        out=YH,
                        in_=PH.rearrange("p (h j) -> p h j", j=J),
                        axis=mybir.AxisListType.X,
                        op=ADD,
                    )
                    T2H = sm.tile([CG, H], F32, name="T2H", tag="T2H")
                    nc.scalar.activation(out=T2H, in_=YH, func=Relu, bias=bact4)
                    T3H = sm.tile([CG, H], F32, name="T3H", tag="T3H")
                    nc.scalar.activation(out=T3H, in_=T2H, func=Copy, bias=-3.0)
                    HSH = sm.tile([CG, H], F16, name="HSH", tag="HSH")
                    nc.vector.scalar_tensor_tensor(
                        out=HSH, in0=T2H, scalar=6.0, in1=T3H, op0=MIN, op1=MULT
                    )
                    for i in range(grp):
                        b = 64 * i
                        nc.tensor.matmul(
                            APs2[:, i : i + 1, 0:H],
                            whw4[b : b + MIP, 0:CG],
                            HSH[b : b + MIP, :].unsqueeze(1),
                            start=True,
                            stop=True,
                            tile_position=(b, 0),
                        )
                    AHE2 = sm.tile([CG, 2, H, 2], F16, name="AHE2", tag="AHE2")
                    nc.scalar.activation(
                        out=AHE2,
                        in_=APs2[:, :, 0:H].unsqueeze(3).broadcast_to([CG, 2, H, 2]),
                        func=Sigmoid,
                        bias=bh,
                    )

                # first TT1 as early as possible
                OUT = op.tile([CG, HW], F16, name="OUT")
                OUTs.append(OUT)
                Xr4 = Xs[0].rearrange("p (h r two) -> p h r two", h=H, two=2)
                OUTr4 = OUT.rearrange("p (h r two) -> p h r two", h=H, two=2)
                ahe_b = AHE2[:, 0].unsqueeze(2).broadcast_to([CG, H, W // 2, 2])
                nc.vector.tensor_tensor(out=OUTr4, in0=Xr4, in1=ahe_b, op=MULT)

                with tc.high_priority(offset=60):
                    # w-chain (runs under TT1)
                    YW = sm.tile([CG, W], F32, name="YW", tag="YW")
                    nc.vector.tensor_reduce(
                        out=YW,
                        in_=PW.rearrange("p (j w) -> p w j", j=J),
                        axis=mybir.AxisListType.X,
                        op=ADD,
                    )
                    T2W = sm.tile([CG, W], F32, name="T2W", tag="T2W")
                    nc.scalar.activation(out=T2W, in_=YW, func=Relu, bias=bact4)
                    T3W = sm.tile([CG, W], F32, name="T3W", tag="T3W")
                    nc.scalar.activation(out=T3W, in_=T2W, func=Copy, bias=-3.0)
                    HSW = sm.tile([CG, W], F16, name="HSW", tag="HSW")
                    nc.vector.scalar_tensor_tensor(
                        out=HSW, in0=T2W, scalar=6.0, in1=T3W, op0=MIN, op1=MULT
                    )
                    for i in range(grp):
                        b = 64 * i
                        nc.tensor.matmul(
                            APs2[:, i : i + 1, H : 2 * H],
                            whw4[b : b + MIP, CG:],
                            HSW[b : b + MIP, :].unsqueeze(1),
                            start=True,
                            stop=True,
                            tile_position=(b, 0),
                        )
                    AW2 = sm.tile([CG, 2, W], F16, name="AW2", tag="AW2")
                    nc.scalar.activation(
                        out=AW2, in_=APs2[:, :, H : 2 * H], func=Sigmoid, bias=bw
                    )

                # remaining TT1s, then TT2 + store per slice
                for i in range(1, grp):
                    OUT = op.tile([CG, HW], F16, name="OUT")
                    OUTs.append(OUT)
                    Xr4 = Xs[i].rearrange("p (h r two) -> p h r two", h=H, two=2)
                    OUTr4 = OUT.rearrange("p (h r two) -> p h r two", h=H, two=2)
                    ahe_b = AHE2[:, i].unsqueeze(2).broadcast_to([CG, H, W // 2, 2])
                    nc.vector.tensor_tensor(out=OUTr4, in0=Xr4, in1=ahe_b, op=MULT)
                for i in range(grp):
                    OUT = OUTs[i]
                    OUTr = OUT.rearrange("p (h w) -> p h w", h=H)
                    aw_b = AW2[:, i].unsqueeze(1).broadcast_to([CG, H, W])
                    nc.vector.tensor_tensor(out=OUTr, in0=OUTr, in1=aw_b, op=MULT)
                    nc.scalar.dma_start(out=out_d[s + i], in_=OUT)
                s += grp

    nc.finalize()
    return nc


def _get_nc():
    global _NC_CACHE
    if _NC_CACHE is None:
        _NC_CACHE = _build_bass()
    return _NC_CACHE


def _prep_weights(W1, b1, gamma, beta, mean, var, Wh, bh, Ww, bw):
    W1 = np.asarray(W1, np.float64)
    b1 = np.asarray(b1, np.float64)
    gamma = np.asarray(gamma, np.float64)
    beta = np.asarray(beta, np.float64)
    mean = np.asarray(mean, np.float64)
    var = np.asarray(var, np.float64)
    Wh = np.asarray(Wh, np.float64)
    Ww = np.asarray(Ww, np.float64)
    bh = np.asarray(bh, np.float64)
    bw = np.asarray(bw, np.float64)

    scale = gamma / np.sqrt(var + EPS)                    # (MIP,)
    w1eff = (W1 * scale[:, None]) / float(W)              # (MIP, CG); mean 1/64
    b1eff = scale * (b1 - mean) + beta                    # (MIP,)

    whw = np.concatenate([(Wh / 6.0).T, (Ww / 6.0).T], axis=1)        # (MIP, 2CG)
    whwc = np.zeros((CG, 2 * CG + MIP), np.float16)
    bact4 = np.zeros((CG, 1), np.float32)
    for b in range(0, CG, 32):
        whwc[b : b + MIP, 0 : 2 * CG] = whw.astype(np.float16)
        bact4[b : b + MIP, 0] = (b1eff + 3.0).astype(np.float32)
    whwc[:, 2 * CG :] = w1eff.T.astype(np.float16)        # (CG, MIP)
    bhw = np.ascontiguousarray(
        np.stack([bh, bw], axis=1).astype(np.float32)
    )                                                     # (CG, 2)
    return whwc, bact4, bhw


def run(inputs: dict, trace: bool = False):
    """Run on 8 NeuronCores. Returns (out [16,512,64,64] fp32, results)."""
    x = np.asarray(inputs["x"], dtype=np.float32)
    n = x.shape[0]
    assert x.shape == (n, C, H, W) and n == N_CORES * NB, x.shape

    whwc, bact4, bhw = _prep_weights(
        inputs["W1"], inputs["b1"], inputs["gamma"], inputs["beta"],
        inputs["mean"], inputs["var"], inputs["Wh"], inputs["bh"],
        inputs["Ww"], inputs["bw"],
    )

    # fp16, pre-sliced per core: [core, slice(b,g), 128, 4096]
    x16 = np.ascontiguousarray(
        x.astype(np.float16).reshape(N_CORES, S, CG, HW)
    )

    nc = _get_nc()
    core_ids = list(range(N_CORES))
    in_maps = []
    for k in core_ids:
        in_maps.append(
            {
                "x": x16[k],
                "whwc": whwc,
                "bact4": bact4,
                "bhw": bhw,
            }
        )

    res = run_bass_kernel_spmd(nc, in_maps, core_ids, trace=trace)
    out16 = np.stack([res.results[k]["out"] for k in core_ids])  # (8,8,128,HW)
    # group-major == natural channel order; then apply the channel shuffle
    # c' = (c % 4) * 128 + c // 4 on the host, with the fp16->fp32 upcast.
    nat = out16.astype(np.float32).reshape(n, C, H, W)
    out = np.ascontiguousarray(
        nat.reshape(n, CG, G, H, W).transpose(0, 2, 1, 3, 4).reshape(n, C, H, W)
    )
    return out, res


def kernel(**inputs) -> np.ndarray:
    out, _ = run(inputs, trace=False)
    return out


def exec_time_ns(res):
    return res.exec_time_ns
